# revision 36
# baseline (speedup 1.0000x reference)
"""Multi-head attention (B=4,S=2048,D=1024,H=16,Hd=64, fp32) on 8 TRN2 NeuronCores.

Sharding: core c handles batch b=c//2 and query-row half h=c%2 (1024 rows).
Each core computes K/V for its full batch (2048 keys), Q for its 1024 rows,
full 16-head attention for those rows, and the output projection. No
collectives; the host gathers per-core [1024,1024] output^T slices. The host
rotates each core's x^T so the core's own rows sit in columns 0-1023 (key
order is irrelevant: attention reduces over keys), keeping the program SPMD.

Schedule: the PE must never idle (an idle gap drops it from 2.4GHz to the
1.2GHz mid p-state for ~3us). Keys run in 4 blocks of 512; the K/V
projection matmuls of block b+1 are interleaved into block b's attention
pair loop so the ACT-bound exp stretches stay filled with PE work. Scores
accumulate into 2-bank [128,1024] PSUM super-tiles (two key tiles per head)
so one exp instruction covers 1024 columns. Ctx accumulates in PSUM across
the block's 4 key tiles (a ones-column in V_aug yields softmax sums as row
64), then drains to an SBUF fp32 accumulator. No max-subtraction (scores
are O(5); exp is fp32-safe). attn*V runs in bf16 (attn<=1, V~N(0,1): ~0.5%
err); scores/projections in f32r; ctx/Wo in bf16. Weights stream as
8KB-per-partition contiguous tiles from a host-side [128,4,8,256] layout.
"""
import numpy as np
import ml_dtypes
from contextlib import ExitStack

import concourse.bass as bass
import concourse.tile as tile
from concourse import bacc, mybir
from concourse.bass import ts, ds
from concourse.bass_utils import run_bass_kernel_spmd

P = 128
D = 1024
KC = 8                 # contraction chunks of 128
S = 2048               # keys per batch
R = 1024               # query rows per core
NB = 4                 # key blocks
SBK = S // NB          # 512 keys per block
KTB = SBK // P         # 4 key tiles per block
H = 16
HP = H // 2            # 8 head pairs
HD = 64
MP = 4                 # weight DMA tiles of 256 output-cols
F32R = mybir.dt.float32r
BF16 = mybir.dt.bfloat16
F32 = mybir.dt.float32
FP = mybir.ActivationFunctionType

_CACHED = {}


def build():
    if "nc" in _CACHED:
        return _CACHED["nc"]
    nc = bacc.Bacc("TRN2", target_bir_lowering=False, debug=False, num_devices=8)
    xTt = nc.dram_tensor("xTt", [P, 8, KC, 256], F32R, kind="ExternalInput").ap()
    Wq4 = nc.dram_tensor("Wq4", [P, MP, KC, 256], F32R, kind="ExternalInput").ap()
    Wk4 = nc.dram_tensor("Wk4", [P, MP, KC, 256], F32R, kind="ExternalInput").ap()
    Wv4 = nc.dram_tensor("Wv4", [P, MP, KC, 256], F32R, kind="ExternalInput").ap()
    Wo4 = nc.dram_tensor("Wo4", [P, MP, KC, 256], BF16, kind="ExternalInput").ap()
    bq = nc.dram_tensor("bq", [P, KC], F32, kind="ExternalInput").ap()
    bk = nc.dram_tensor("bk", [P, KC], F32, kind="ExternalInput").ap()
    bo = nc.dram_tensor("bo", [P, KC], F32, kind="ExternalInput").ap()
    bvb = nc.dram_tensor("bvb", [P, D], F32, kind="ExternalInput").ap()
    outT = nc.dram_tensor("outT", [D, R], BF16, kind="ExternalOutput").ap()


    with tile.TileContext(nc) as tc:
        with ExitStack() as ctx:
            const = ctx.enter_context(tc.tile_pool(name="const", bufs=1))
            wpool = ctx.enter_context(tc.tile_pool(name="wpool", bufs=3))
            xpool = ctx.enter_context(tc.tile_pool(name="xpool", bufs=3))
            qtp = ctx.enter_context(tc.tile_pool(name="qtp", bufs=1))
            ktp = ctx.enter_context(tc.tile_pool(name="ktp", bufs=2))
            vgp = ctx.enter_context(tc.tile_pool(name="vgp", bufs=2))
            accp = ctx.enter_context(tc.tile_pool(name="accp", bufs=1))
            ctp = ctx.enter_context(tc.tile_pool(name="ctp", bufs=1))
            attnp = ctx.enter_context(tc.tile_pool(name="attnp", bufs=6))
            bcp = ctx.enter_context(tc.tile_pool(name="bcp", bufs=2))
            outp = ctx.enter_context(tc.tile_pool(name="outp", bufs=2))
            sps = ctx.enter_context(tc.tile_pool(name="sps", bufs=2, space="PSUM"))
            cps = ctx.enter_context(tc.tile_pool(name="cps", bufs=2, space="PSUM"))
            
            # ---- constants ----
            bq_t = const.tile([P, KC], F32, tag="bq")
            nc.sync.dma_start(bq_t[:], bq)
            bk_t = const.tile([P, KC], F32, tag="bk")
            nc.sync.dma_start(bk_t[:], bk)
            bo_t = const.tile([P, KC], F32, tag="bo")
            nc.sync.dma_start(bo_t[:], bo)
            bv_bc = const.tile([P, D], F32, tag="bvb")
            nc.sync.dma_start(bv_bc[:], bvb)

            QT = qtp.tile([P, KC, R], BF16, tag="qt")
            CT = ctp.tile([P, KC, R], BF16, tag="ct")
            # 64 ctx rows + softmax-sum row per head (all partition-0 based:
            # multi-input DVE ops require inputs to share a start partition)
            ctxacc = accp.tile([65, H, 2, 512], F32, tag="acc")

            # ---------- Q^T projection (rows = xT cols 0..1023) ----------
            # psum [128,1024] = two m2 banks x two g quarters per bank.
            # Within a bank only the first mm has start=True (start clears
            # has_written for the whole bank; cleared bits mean overwrite,
            # set bits mean accumulate), k outer so each stationary serves
            # two matmuls (hides LDWEIGHTS).
            for rt in range(2):
                xq = []
                for g in range(2):
                    xg = xpool.tile([P, KC, 256], F32R, tag="x")
                    nc.sync.dma_start(xg[:], xTt[:, rt * 2 + g])
                    xq.append(xg)
                for mp in range(MP):
                    wq = wpool.tile([P, KC, 256], F32R, tag="w")
                    nc.sync.dma_start(wq[:], Wq4[:, mp])
                    ps = sps.tile([P, 1024], F32, tag="sp")
                    for m2 in range(2):
                        for k in range(KC):
                            for g in range(2):
                                nc.tensor.matmul(
                                    ps[:, ds(m2 * 512 + g * 256, 256)],
                                    wq[:, k, ts(m2, P)], xq[g][:, k],
                                    start=(k == 0 and g == 0),
                                    stop=(k == KC - 1 and g == 1),
                                    skip_group_check=True)
                    for m2 in range(2):
                        m = 2 * mp + m2
                        nc.vector.tensor_scalar_add(
                            QT[:, m, ts(rt, 512)], ps[:, ts(m2, 512)],
                            bq_t[:, m:m + 1])

            # ---------- K/V projection units (yield per matmul) ----------
            def gen_k_unit(kt_tile, xb, wk, mp):
                # K^T for one weight m-pair over this block's 512 keys
                ps = sps.tile([P, 1024], F32, tag="sp")
                for m2 in range(2):
                    for k in range(KC):
                        for g in range(2):
                            nc.tensor.matmul(
                                ps[:, ds(m2 * 512 + g * 256, 256)],
                                wk[:, k, ts(m2, P)], xb[g][:, k],
                                start=(k == 0 and g == 0),
                                stop=(k == KC - 1 and g == 1),
                                skip_group_check=True)
                            yield
                for m2 in range(2):
                    m = 2 * mp + m2
                    nc.vector.tensor_scalar_add(
                        kt_tile[:, m, :], ps[:, ts(m2, 512)], bk_t[:, m:m + 1])

            def gen_v_unit(vaug, xb, wv2, ktp):
                # V (natural) for two key tiles x one 512-wide v-col pair
                ntp = wv2[2]
                ps = sps.tile([P, 1024], F32, tag="sp")
                for kh in range(2):
                    kt = 2 * ktp + kh
                    for k in range(KC):
                        for hh in range(2):
                            nc.tensor.matmul(
                                ps[:, ds(kh * 512 + hh * 256, 256)],
                                xb[kt // 2][:, k, ts(kt % 2, P)],
                                wv2[hh][:, k],
                                start=(k == 0 and hh == 0),
                                stop=(k == KC - 1 and hh == 1),
                                skip_group_check=True)
                            yield
                h0 = ntp * 8
                for kh in range(2):
                    kt = 2 * ktp + kh
                    vdst = vaug[:, kt, :].rearrange(
                        "p (h c) -> p h c", c=65)[:, h0:h0 + 8, 0:64]
                    nc.vector.tensor_tensor(
                        vdst,
                        ps[:, ts(kh, 512)].rearrange("p (h c) -> p h c", c=HD),
                        bv_bc[:, ds(ntp * 512, 512)].rearrange(
                            "p (h c) -> p h c", c=HD),
                        mybir.AluOpType.add)

            def write_ones(vaug):
                ones_view = vaug[:].rearrange(
                    "p kt (h c) -> p kt h c", c=65)[:, :, :, 64:65]
                nc.vector.tensor_scalar(
                    ones_view,
                    bv_bc[:, 0:KTB * H].rearrange(
                        "p (kt h) -> p kt h", kt=KTB).unsqueeze(3),
                    0.0, 1.0, mybir.AluOpType.mult, mybir.AluOpType.add)

            def make_kv_tiles(b):
                kt_tile = ktp.tile([P, KC, SBK], BF16, tag="kt", name=f"KT{b}")
                vaug = vgp.tile([P, KTB, H * 65], BF16, tag="vg", name=f"VG{b}")
                return kt_tile, vaug

            def load_xb(b):
                xb = []
                for g in range(2):
                    xg = xpool.tile([P, KC, 256], F32R, tag="x", name=f"xb{b}_{g}")
                    nc.sync.dma_start(xg[:], xTt[:, b * 2 + g])
                    xb.append(xg)
                return xb

            def gen_proj_unit(b1, i):
                # one projection unit for block b1, slotted at hp index i
                if i < 4:
                    mp = i
                    wk = wpool.tile([P, KC, 256], F32R, tag="w",
                                    name=f"wk{b1}_{mp}")
                    nc.sync.dma_start(wk[:], Wk4[:, mp])
                    yield from gen_k_unit(kv[b1][0], xb_next, wk, mp)
                else:
                    j = i - 4
                    ntp, ktp2 = j // 2, j % 2
                    if ktp2 == 0:
                        wv2 = []
                        for hh in range(2):
                            wv = wpool.tile([P, KC, 256], F32R, tag="w",
                                            name=f"wv{b1}_{ntp}{hh}")
                            nc.sync.dma_start(wv[:], Wv4[:, 2 * ntp + hh])
                            wv2.append(wv)
                        wv2.append(ntp)
                        wv_state[0] = wv2
                    yield from gen_v_unit(kv[b1][1], xb_next, wv_state[0], ktp2)
                    if ntp == 1 and ktp2 == 1:
                        write_ones(kv[b1][1])

            # ---------- block 0 K/V projection (no attention to overlap) ----
            kv = [None] * NB
            kv[0] = make_kv_tiles(0)
            xb0 = load_xb(0)
            for mp in range(MP):
                wk = wpool.tile([P, KC, 256], F32R, tag="w", name=f"wk0_{mp}")
                nc.sync.dma_start(wk[:], Wk4[:, mp])
                for _ in gen_k_unit(kv[0][0], xb0, wk, mp):
                    pass
            for ntp in range(2):
                wv2 = []
                for hh in range(2):
                    wv = wpool.tile([P, KC, 256], F32R, tag="w", name=f"wv0_{ntp}{hh}")
                    nc.sync.dma_start(wv[:], Wv4[:, 2 * ntp + hh])
                    wv2.append(wv)
                wv2.append(ntp)
                for ktp2 in range(2):
                    for _ in gen_v_unit(kv[0][1], xb0, wv2, ktp2):
                        pass
            write_ones(kv[0][1])

            # ---------- attention: hp slots, both rt, proj(b+1) woven in ----
            wo_tiles = {}

            def gen_scores(bb, hp, supers):
                # per (head, kt-pair): two supers (rt0, rt1); each stationary
                # (head, kt) serves both rt matmuls back-to-back
                kt_tile = kv[bb][0]
                for head in range(2):
                    po = 64 * head
                    for p2 in range(2):
                        sup = [sps.tile([P, 1024], F32, tag="sp",
                                        name=f"sup{head}{p2}r{rt}")
                               for rt in range(2)]
                        for kh in range(2):
                            kt = 2 * p2 + kh
                            for rt in range(2):
                                nc.tensor.matmul(
                                    sup[rt][:, ts(kh, 512)],
                                    kt_tile[po:po + 64, hp, ts(kt, P)],
                                    QT[po:po + 64, hp, ts(rt, 512)],
                                    start=True, stop=True,
                                    tile_position=(po, 0))
                                yield
                        for rt in range(2):
                            at = attnp.tile([P, 1024], BF16, tag="a")
                            nc.scalar.activation(at[:], sup[rt][:], FP.Exp,
                                                 scale=0.125)
                            supers.append(at)

            def gen_ctx(bb, hp, supers):
                vaug = kv[bb][1]
                for head in range(2):
                    h = 2 * hp + head
                    po = head * 64
                    cp = cps.tile([65, 1024], F32, tag="cp")
                    for kt in range(KTB):
                        for rt in range(2):
                            at = supers[head * 4 + (kt // 2) * 2 + rt]
                            nc.tensor.matmul(
                                cp[:, ts(rt, 512)],
                                vaug[:, kt, ds(h * 65, 65)],
                                at[:, ts(kt % 2, 512)],
                                start=(kt == 0), stop=(kt == KTB - 1))
                            yield
                    for rt in range(2):
                        if bb == 0:
                            nc.vector.tensor_copy(ctxacc[:, h, rt],
                                                  cp[:, ts(rt, 512)])
                        else:
                            nc.vector.tensor_add(ctxacc[:, h, rt],
                                                 ctxacc[:, h, rt],
                                                 cp[:, ts(rt, 512)])
                        if bb == NB - 1:
                            # normalize into CT; approx_fast needs contiguous
                            # input: stage sums through bc row 0
                            rec = bcp.tile([1, 512], F32, tag="rec")
                            bc = bcp.tile([64, 512], F32, tag="bc")
                            nc.vector.tensor_copy(bc[0:1], ctxacc[64:65, h, rt])
                            nc.vector.reciprocal_approx_fast(rec[:], bc[0:1])
                            nc.gpsimd.partition_broadcast(bc[:], rec[:])
                            nc.vector.tensor_mul(
                                CT[po:po + 64, h // 2, ts(rt, 512)],
                                ctxacc[0:64, h, rt], bc[:])

            def interleave(gens):
                # round-robin one PE instruction at a time
                gens = [g for g in gens if g is not None]
                while gens:
                    alive = []
                    for g in gens:
                        try:
                            next(g)
                            alive.append(g)
                        except StopIteration:
                            pass
                    gens = alive

            pending = []   # (bb, hp, supers) awaiting ctx; LAG 1 slot
            wv_state = [None]
            for b in range(NB):
                if b + 1 < NB:
                    kv[b + 1] = make_kv_tiles(b + 1)
                    xb_next = load_xb(b + 1)
                for hp in range(HP):
                    supers = []
                    pending.append((b, hp, supers))
                    gsc = gen_scores(b, hp, supers)
                    gpr = gen_proj_unit(b + 1, hp) if b + 1 < NB else None
                    gcx = None
                    if len(pending) > 1:
                        gcx = gen_ctx(*pending.pop(0))
                    if b == NB - 1 and 1 <= hp < 5:
                        # prefetch Wo during the last block
                        mp = hp - 1
                        wo = wpool.tile([P, KC, 256], BF16, tag="w",
                                        name=f"wo_{mp}")
                        nc.sync.dma_start(wo[:], Wo4[:, mp])
                        wo_tiles[mp] = wo
                    interleave([gsc, gpr, gcx])
            for ent in pending:
                for _ in gen_ctx(*ent):
                    pass

            # ---------- out^T = (ctx @ Wo)^T + bo ----------
            for mp in range(MP):
                wo = wo_tiles[mp]
                for rt in range(2):
                    ps = sps.tile([P, 1024], F32, tag="sp")
                    for m2 in range(2):
                        for k in range(KC):
                            nc.tensor.matmul(
                                ps[:, ts(m2, 512)], wo[:, k, ts(m2, P)],
                                CT[:, k, ts(rt, 512)],
                                start=(k == 0), stop=(k == KC - 1))
                    for m2 in range(2):
                        m = 2 * mp + m2
                        ob = outp.tile([P, 512], BF16, tag="ob")
                        nc.vector.tensor_scalar_add(ob[:], ps[:, ts(m2, 512)],
                                                    bo_t[:, m:m + 1])
                        nc.sync.dma_start(outT[ts(m, P), ts(rt, 512)], ob[:])

    nc.compile()
    _CACHED["nc"] = nc
    return nc


def make_in_maps(x, Wq, bq, Wk, bk, Wv, bv, Wo, bo):
    x = np.asarray(x, dtype=np.float32)
    B = x.shape[0]

    def bcol(b):
        return np.ascontiguousarray(np.asarray(b, np.float32).reshape(KC, P).T)

    def w4(w, dt=np.float32):
        w = np.asarray(w, np.float32).reshape(KC, P, MP, 256)
        return np.ascontiguousarray(w.transpose(1, 2, 0, 3).astype(dt))

    wq4, wk4, wv4 = w4(Wq), w4(Wk), w4(Wv)
    wo4 = w4(Wo, ml_dtypes.bfloat16)
    bq2, bk2, bo2 = bcol(bq), bcol(bk), bcol(bo)
    bv1 = np.ascontiguousarray(np.asarray(bv, np.float32).reshape(1, D))

    bvb = np.ascontiguousarray(np.tile(bv1, (P, 1)))
    in_maps = []
    for c in range(8):
        b, half = c // 2, c % 2
        xb = x[b]
        if half == 1:
            xb = np.concatenate([xb[R:], xb[:R]], axis=0)
        # tiled x^T: [p, tile(256 rows), kc, j] with 8KB-contiguous partitions
        xtt = np.ascontiguousarray(
            xb.reshape(8, 256, KC, P).transpose(3, 0, 2, 1))
        in_maps.append({
            "xTt": xtt,
            "Wq4": wq4, "Wk4": wk4, "Wv4": wv4, "Wo4": wo4,
            "bq": bq2, "bk": bk2, "bo": bo2, "bvb": bvb,
        })
    return in_maps


def assemble_out(results, B):
    out = np.empty((B, S, D), dtype=np.float32)
    for c in range(8):
        b, half = c // 2, c % 2
        out[b, half * R:(half + 1) * R, :] = results[c]["outT"].T.astype(np.float32)
    return out


def kernel(x, Wq, bq, Wk, bk, Wv, bv, Wo, bo, **kw):
    nc = build()
    in_maps = make_in_maps(x, Wq, bq, Wk, bk, Wv, bv, Wo, bo)
    res = run_bass_kernel_spmd(nc, in_maps, core_ids=list(range(8)))
    return assemble_out(res.results, np.asarray(x).shape[0])
